# revision 6
# baseline (speedup 1.0000x reference)
"""Trainium2 Bass kernel for single-token KV-cache attention (decode step).

Reference computation (B=16, T=1, NX=2048, H=16, DH=128, KV=4096):
  c = x @ w_attn + b_attn; q,k,v = split(c)
  k_new/v_new = cache with new k/v inserted at position `length`
  attn over the first `length` cache positions only (the new token itself is
  masked out by the reference's length-based mask)
  out = merge_heads(attn) @ w_proj + b_proj
Returns (out, k_new, v_new) exactly like the reference.

Sharding: tensor-parallel over heads. 16 heads / 8 cores = 2 heads per core.
Each core receives its head-slice of the KV caches, the matching column-slice
of w_attn and row-slice of w_proj, plus the full x (replicated). The output
projection is row-parallel, so each core emits a partial [16, 2048] output;
the host sums the 8 partials and adds b_proj (the TP unshard step).
"""

import numpy as np

import concourse.bass as bass
import concourse.tile as tile
from concourse import mybir
from concourse.bass_utils import run_bass_kernel_spmd
from concourse.masks import make_identity

B = 16
NX = 2048
H = 16
DH = 128
KV = 4096
N_CORES = 8
HPC = H // N_CORES          # heads per core = 2
NPAIR = B * HPC             # (head, batch) pairs per core = 32
F32 = mybir.dt.float32
SCALE = 1.0 / float(np.sqrt(DH))


def _build(length: int):
    """Build the SPMD Bass program with `length` baked in."""
    assert 1 <= length < KV
    cfull, rem = divmod(length, DH)
    nch = cfull + (1 if rem else 0)   # 128-wide cache chunks to attend over

    nc = bass.Bass()
    x2 = nc.declare_dram_parameter("x2", [B, NX], F32, isOutput=False)
    wqkv = nc.declare_dram_parameter("wqkv", [NX, 3 * HPC * DH], F32, isOutput=False)
    bqkv = nc.declare_dram_parameter("bqkv", [B, 3 * HPC * DH], F32, isOutput=False)
    wproj = nc.declare_dram_parameter("wproj", [HPC * DH, NX], F32, isOutput=False)
    memk = nc.declare_dram_parameter("memk", [B, HPC, DH, KV], F32, isOutput=False)
    memv = nc.declare_dram_parameter("memv", [B, HPC, KV, DH], F32, isOutput=False)
    kout = nc.declare_dram_parameter("kout", [B, HPC, DH, KV], F32, isOutput=True)
    vout = nc.declare_dram_parameter("vout", [B, HPC, KV, DH], F32, isOutput=True)
    pout = nc.declare_dram_parameter("pout", [B, NX], F32, isOutput=True)

    FQ = 3 * HPC * DH  # 768 fused qkv width per core

    with tile.TileContext(nc) as tc:
        with (
            tc.tile_pool(name="const", bufs=1) as cpool,
            tc.tile_pool(name="kv", bufs=3) as kpool,
            tc.tile_pool(name="vv", bufs=3) as vpool,
            tc.tile_pool(name="exp", bufs=4) as epool,
            tc.tile_pool(name="pA", bufs=1, space="PSUM") as pA,
        ):
            ident = cpool.tile([B, B], F32)
            make_identity(nc, ident[:])
            ones = cpool.tile([DH, 1], F32)
            nc.vector.memset(ones[:], 1.0)
            ones_row = cpool.tile([1, DH], F32)
            nc.vector.memset(ones_row[:], 1.0)

            wqkv_sb = cpool.tile([DH, NX // DH, FQ], F32)
            nc.sync.dma_start(wqkv_sb[:], wqkv[:].rearrange("(o p) f -> p o f", p=DH))
            wproj_sb = cpool.tile([DH, HPC, NX], F32)
            nc.sync.dma_start(wproj_sb[:], wproj[:].rearrange("(o p) f -> p o f", p=DH))
            bias_sb = cpool.tile([B, FQ], F32)
            nc.sync.dma_start(bias_sb[:], bqkv[:])
            x_sb = cpool.tile([B, NX], F32)
            nc.sync.dma_start(x_sb[:], x2[:])

            xT_sb = cpool.tile([DH, NX // DH, B], F32)
            c_sb = cpool.tile([B, FQ], F32)
            qkT_sb = cpool.tile([DH, 2 * NPAIR], F32)
            rowsum = cpool.tile([DH, NPAIR], F32)
            a_psum = pA.tile([DH, NPAIR], F32)

            # ---- prologue: x transpose + fused QKV projection -------------
            with (
                tc.tile_pool(name="pT", bufs=2, space="PSUM") as pT,
                tc.tile_pool(name="pC", bufs=1, space="PSUM") as pC,
            ):
                for o in range(NX // DH):
                    tp = pT.tile([DH, B], F32)
                    nc.tensor.transpose(tp[:], x_sb[:, o * DH:(o + 1) * DH], ident[:])
                    nc.scalar.copy(xT_sb[:, o, :], tp[:])

                c_psum = pC.tile([B, FQ], F32)
                for n0 in range(0, FQ, 512):
                    n1 = min(n0 + 512, FQ)
                    for o in range(NX // DH):
                        nc.tensor.matmul(
                            c_psum[:, n0:n1],
                            lhsT=xT_sb[:, o, :],
                            rhs=wqkv_sb[:, o, n0:n1],
                            start=(o == 0),
                            stop=(o == NX // DH - 1),
                        )
                nc.vector.tensor_tensor(
                    c_sb[:], c_psum[:], bias_sb[:], mybir.AluOpType.add
                )

                # transpose q and k blocks -> [dh, pair] columns
                for j in range(2 * HPC):  # q_h0, q_h1, k_h0, k_h1
                    tp = pT.tile([DH, B], F32)
                    nc.tensor.transpose(
                        tp[:], c_sb[:, j * DH:(j + 1) * DH], ident[:]
                    )
                    nc.scalar.copy(qkT_sb[:, j * B:(j + 1) * B], tp[:])

            # ---- main loop over (head, batch) pairs -----------------------
            with tc.tile_pool(name="pS", bufs=2, space="PSUM") as pS:
                for hi in range(HPC):
                    voff = (2 * HPC + hi) * DH
                    for b in range(B):
                        pair = hi * B + b

                        kt = kpool.tile([DH, KV], F32)
                        nc.sync.dma_start(kt[:], memk[b, hi])
                        # insert new k column at position `length`
                        # (k columns live at qkT_sb[:, NPAIR + pair])
                        nc.vector.tensor_copy(
                            kt[:, length:length + 1],
                            qkT_sb[:, NPAIR + pair:NPAIR + pair + 1],
                        )
                        nc.scalar.dma_start(kout[b, hi], kt[:])

                        sc = pS.tile([DH, nch], F32)
                        for c in range(nch):
                            nc.tensor.matmul(
                                sc[:, c:c + 1],
                                lhsT=kt[:, c * DH:(c + 1) * DH],
                                rhs=qkT_sb[:, pair:pair + 1],
                                start=True,
                                stop=True,
                            )
                        if rem:
                            nc.vector.memset(sc[rem:, cfull:cfull + 1], -1e30)
                        ex = epool.tile([DH, nch], F32)
                        nc.scalar.activation(
                            ex[:],
                            sc[:],
                            mybir.ActivationFunctionType.Exp,
                            scale=SCALE,
                            accum_out=rowsum[:, pair:pair + 1],
                        )

                        vt = vpool.tile([DH, KV // DH, DH], F32)
                        nc.sync.dma_start(
                            vt[:], memv[b, hi].rearrange("(c p) d -> p c d", p=DH)
                        )
                        # insert new v row at position `length`
                        nc.sync.dma_start(
                            vt[length % DH:length % DH + 1, length // DH, :],
                            c_sb[b:b + 1, voff:voff + DH],
                        )
                        nc.scalar.dma_start(
                            vout[b, hi].rearrange("(c p) d -> p c d", p=DH), vt[:]
                        )

                        for c in range(nch):
                            nc.tensor.matmul(
                                a_psum[:, pair:pair + 1],
                                lhsT=vt[:, c, :],
                                rhs=ex[:, c:c + 1],
                                start=(c == 0),
                                stop=(c == nch - 1),
                            )

            # ---- softmax normalization + output projection ----------------
            with (
                tc.tile_pool(name="pD", bufs=1, space="PSUM") as pD,
                tc.tile_pool(name="pO", bufs=1, space="PSUM") as pO,
            ):
                den_ps = pD.tile([1, NPAIR], F32)
                nc.tensor.matmul(
                    den_ps[:], lhsT=ones[:], rhs=rowsum[:], start=True, stop=True
                )
                recip = cpool.tile([1, NPAIR], F32)
                nc.vector.reciprocal(recip[:], den_ps[:])
                # broadcast recip across all 128 partitions: ones ⊗ recip
                recipb_ps = pD.tile([DH, NPAIR], F32)
                nc.tensor.matmul(
                    recipb_ps[:], lhsT=ones_row[:], rhs=recip[:],
                    start=True, stop=True,
                )
                recipb = cpool.tile([DH, NPAIR], F32)
                nc.scalar.copy(recipb[:], recipb_ps[:])
                a_sb = cpool.tile([DH, NPAIR], F32)
                nc.vector.tensor_tensor(
                    a_sb[:], a_psum[:], recipb[:], mybir.AluOpType.mult
                )

                out_ps = pO.tile([B, NX], F32)
                for n0 in range(0, NX, 512):
                    for hi in range(HPC):
                        nc.tensor.matmul(
                            out_ps[:, n0:n0 + 512],
                            lhsT=a_sb[:, hi * B:(hi + 1) * B],
                            rhs=wproj_sb[:, hi, n0:n0 + 512],
                            start=(hi == 0),
                            stop=(hi == HPC - 1),
                        )
                out_sb = cpool.tile([B, NX], F32)
                nc.scalar.copy(out_sb[:], out_ps[:])
                nc.sync.dma_start(pout[:], out_sb[:])

    _split_excess_waits(nc)
    return nc


def _split_excess_waits(nc, max_waits: int = 1):
    """walrus here allows only one sync-wait per instruction; move extras
    onto NoOps inserted just before (same engine, same blocking effect)."""
    for f in nc.m.functions:
        for bb in f.blocks:
            new_insts = []
            for inst in bb.instructions:
                si = getattr(inst, "sync_info", None)
                waits = list(si.on_wait) if si and si.on_wait else []
                if len(waits) > max_waits:
                    extra, keep = waits[:-max_waits], waits[-max_waits:]
                    while extra:
                        chunk, extra = extra[:max_waits], extra[max_waits:]
                        new_insts.append(
                            mybir.InstNoOp(
                                name=f"{inst.name}-ws{len(new_insts)}",
                                engine=inst.engine,
                                ins=[],
                                outs=[],
                                sync_info=mybir.SyncInfo(
                                    on_update=[], on_wait=chunk
                                ),
                            )
                        )
                    si.on_wait = keep
                new_insts.append(inst)
            bb.instructions = new_insts


_CACHE: dict[int, object] = {}


def _get_program(length: int):
    if length not in _CACHE:
        _CACHE[length] = _build(length)
    return _CACHE[length]


def _run(x, mem_k, mem_v, w_attn, b_attn, w_proj, b_proj, length, trace=False):
    length = int(length)
    x = np.ascontiguousarray(np.asarray(x, np.float32).reshape(B, NX))
    mem_k = np.asarray(mem_k, np.float32)
    mem_v = np.asarray(mem_v, np.float32)
    w_attn = np.asarray(w_attn, np.float32)
    b_attn = np.asarray(b_attn, np.float32)
    w_proj = np.asarray(w_proj, np.float32)
    b_proj = np.asarray(b_proj, np.float32)

    if length == 0 or length >= KV:
        return _numpy_fallback(
            x, mem_k, mem_v, w_attn, b_attn, w_proj, b_proj, length
        )

    nc = _get_program(length)

    in_maps = []
    for i in range(N_CORES):
        h0 = i * HPC
        hs = slice(h0, h0 + HPC)
        cols = np.concatenate(
            [np.arange(h0 * DH + j * NX, (h0 + HPC) * DH + j * NX) for j in range(3)]
        )
        wq = np.ascontiguousarray(w_attn[:, cols])
        bq = np.broadcast_to(b_attn[cols], (B, 3 * HPC * DH)).copy()
        in_maps.append(
            {
                "x2": x,
                "wqkv": wq,
                "bqkv": bq,
                "wproj": np.ascontiguousarray(w_proj[h0 * DH:(h0 + HPC) * DH, :]),
                "memk": np.ascontiguousarray(mem_k[:, hs]),
                "memv": np.ascontiguousarray(mem_v[:, hs]),
            }
        )

    res = run_bass_kernel_spmd(nc, in_maps, list(range(N_CORES)), trace=trace)

    out = np.zeros((B, NX), np.float32)
    k_new = np.empty((B, H, DH, KV), np.float32)
    v_new = np.empty((B, H, KV, DH), np.float32)
    for i in range(N_CORES):
        r = res.results[i]
        out += r["pout"]
        k_new[:, i * HPC:(i + 1) * HPC] = r["kout"]
        v_new[:, i * HPC:(i + 1) * HPC] = r["vout"]
    out = (out + b_proj).reshape(B, 1, NX).astype(np.float32)
    return (out, k_new, v_new), res


def _numpy_fallback(x, mem_k, mem_v, w_attn, b_attn, w_proj, b_proj, length):
    """Host path for degenerate lengths (length==0: everything is masked and
    softmax degenerates to uniform over all KV positions)."""
    assert length == 0, f"unsupported length {length}"
    c = x @ w_attn + b_attn
    _, k, v = np.split(c, 3, axis=1)
    k_new = mem_k.copy()
    v_new = mem_v.copy()
    k_new[:, :, :, 0] = k.reshape(B, H, DH)
    v_new[:, :, 0, :] = v.reshape(B, H, DH)
    a = v_new.mean(axis=2).reshape(B, H * DH)
    out = (a @ w_proj + b_proj).reshape(B, 1, NX).astype(np.float32)
    return (out, k_new.astype(np.float32), v_new.astype(np.float32)), None


def kernel(x, mem_k, mem_v, w_attn, b_attn, w_proj, b_proj, length):
    (out, k_new, v_new), _ = _run(
        x, mem_k, mem_v, w_attn, b_attn, w_proj, b_proj, length
    )
    return out, k_new, v_new


def _ensure_ntff_hook():
    """bass_utils' trace path imports antenv.axon_hooks, which this image's
    antenv package lacks. Register an equivalent module backed by the same
    ctypes hook trn_agent_boot would have installed."""
    try:
        from antenv.axon_hooks import get_axon_ntff_profile_hook  # noqa: F401
        return
    except ImportError:
        pass
    import sys
    import types

    import antenv
    from trn_agent_boot.trn_boot import _ntff_profile_via_ctypes

    hook = _ntff_profile_via_ctypes("/opt/axon/libaxon_pjrt.so")
    mod = types.ModuleType("antenv.axon_hooks")
    mod.get_axon_ntff_profile_hook = lambda: hook
    mod.set_axon_ntff_profile_hook = lambda h: None
    sys.modules["antenv.axon_hooks"] = mod
    antenv.axon_hooks = mod


def kernel_traced(x, mem_k, mem_v, w_attn, b_attn, w_proj, b_proj, length):
    """Like kernel() but also returns the BassKernelResults with profile."""
    _ensure_ntff_hook()
    return _run(
        x, mem_k, mem_v, w_attn, b_attn, w_proj, b_proj, length, trace=True
    )
